# revision 9
# baseline (speedup 1.0000x reference)
"""Trainium2 Bass kernel for CenterWoParamMultiCosineLoss (l2Norm branch).

Contract: kernel(**inputs) takes FULL inputs (x [8192,1024] f32,
labels [8192] i64/i32, centers [90,16,1024] f32) and returns the FULL
output (scalar f32 loss), running on 8 NeuronCores data-parallel over
the batch.

Math (per sample b, with label c = labels[b], K=16 centers per class):
    xn = x / ||x||;  cn = centers / ||centers||  (rows, +1e-12 under sqrt)
    t_k = xn . cn[c,k]                (16 cosine sims)
    d_k = 1 - t_k
    per_sample = sum_k (1 - d_k/sd) * d_k = sd - ssq/sd
      where sd = sum_k d_k,  ssq = sum_k d_k^2
    loss = mean(per_sample)

End-to-end wall time is dominated by the host->device tunnel, so the
host ships as few bytes as possible:
  - x is cast to fp8e4m3 on host (8 MB total, batch-sharded);
    ||x||^2 is computed on host in exact fp32 and shipped as [128,8]
    per core (tiny).
  - centers are normalized on host, cast to fp8, and sharded 180
    rows/core (1.5 MB total); the device runs an AllGather to
    reconstruct the full 1440-row table on every core.
  - constant tables (colck, ident) are committed to the devices once
    at init and reused every call.
  - the jitted shard_map executable is built once and cached (the
    library path rebuilds it per call).

Device per core (1024 samples):
    - AllGather centers shard -> cn [1440,1024] fp8; PE-transpose into
      the matmul layout cnT [128, 8, 1440] fp8.
    - per 128-sample tile: PE-transpose x tile, 12 DoubleRow fp8
      matmuls S[b, ck] for all 1440 (class,k) columns.
    - masked = S * onehot(label-per-column); T_raw = rowsum(masked),
      Q_raw = rowsum(masked^2) via ACT accum_out.
    - tail: T = T_raw/||x||, Q = Q_raw/||x||^2, per_sample = sd-ssq/sd.
    - host sums the 8x[128,8] per-sample values -> mean.
"""

import os
import sys
from contextlib import ExitStack

import numpy as np

for _p in ("/opt/trn_rl_repo", "/root/.axon_site/_ro/trn_rl_repo"):
    if os.path.isdir(_p) and _p not in sys.path:
        sys.path.insert(0, _p)

import ml_dtypes
import jax
import jax.numpy as jnp
from jax.experimental.shard_map import shard_map
from jax.sharding import Mesh, NamedSharding, PartitionSpec as PSpec

import concourse.bacc as bacc
import concourse.tile as tile
from concourse import mybir
from concourse.bass2jax import _bass_exec_p, install_neuronx_cc_hook

N_CORES = 8
B = 8192
B_LOCAL = B // N_CORES  # 1024 samples per core
P = 128                 # partitions
N_TILES = B_LOCAL // P  # 8 sample tiles per core
D = 1024                # feature dim
C = 90                  # classes
K = 16                  # centers per class
CK = C * K              # 1440
CK_LOCAL = CK // N_CORES  # 180 center rows shipped per core
D_CHUNKS = D // P       # 8 contraction chunks
EPS = 1e-12

FP32 = mybir.dt.float32
BF16 = mybir.dt.bfloat16
FP8 = mybir.dt.float8e4

NP_FP8 = ml_dtypes.float8_e4m3
NP_BF16 = ml_dtypes.bfloat16

# AllGather the centers shard on-device (1.5 MB upload) instead of
# replicating the table to all 8 cores from host (12 MB upload).
USE_CC = os.environ.get("BASS_CC", "1") == "1"
LAZY_INIT = os.environ.get("BASS_LAZY", "0") == "1"


def _build_nc(use_cc):
    nc = bacc.Bacc("TRN2", target_bir_lowering=False, debug=False,
                   num_devices=N_CORES)

    xq_dram = nc.dram_tensor("xq", [B_LOCAL, D], FP8, kind="ExternalInput").ap()
    labels_dram = nc.dram_tensor("labels", [P, N_TILES], FP32, kind="ExternalInput").ap()
    ss_dram = nc.dram_tensor("ss", [P, N_TILES], FP32, kind="ExternalInput").ap()
    cn_rows = CK_LOCAL if use_cc else CK
    cnq_dram = nc.dram_tensor("cnq", [cn_rows, D], FP8, kind="ExternalInput").ap()
    colck_dram = nc.dram_tensor("colck", [P, CK], BF16, kind="ExternalInput").ap()
    ident_dram = nc.dram_tensor("ident", [P, P], BF16, kind="ExternalInput").ap()
    out_dram = nc.dram_tensor("out", [P, N_TILES], FP32, kind="ExternalOutput").ap()

    with tile.TileContext(nc) as tc, ExitStack() as ctx:
        singles = ctx.enter_context(tc.tile_pool(name="singles", bufs=1))
        cpool = ctx.enter_context(tc.tile_pool(name="cpool", bufs=3))
        xpool = ctx.enter_context(tc.tile_pool(name="xpool", bufs=4))
        spool = ctx.enter_context(tc.tile_pool(name="spool", bufs=3))
        psum = ctx.enter_context(tc.tile_pool(name="psum", bufs=2, space="PSUM"))

        # ---- constants / per-sample stats ----
        ident = singles.tile([P, P], BF16, tag="ident")
        nc.sync.dma_start(out=ident, in_=ident_dram)
        colck = singles.tile([P, CK], BF16, tag="colck")  # class id per S column
        nc.sync.dma_start(out=colck, in_=colck_dram)
        labels_sb = singles.tile([P, N_TILES], FP32, tag="labels_sb")
        nc.sync.dma_start(out=labels_sb, in_=labels_dram)
        ss_all = singles.tile([P, N_TILES], FP32, tag="ss_all")  # sum x^2 (host)
        nc.sync.dma_start(out=ss_all, in_=ss_dram)
        eps_col = singles.tile([P, 1], FP32, tag="eps_col")
        nc.vector.memset(eps_col, EPS)

        t_all = singles.tile([P, N_TILES], FP32, tag="t_all")    # T_raw
        q_all = singles.tile([P, N_TILES], FP32, tag="q_all")    # Q_raw
        junk_bf = singles.tile([P, CK], BF16, tag="junk_bf")

        # persistent transposed-normalized centers, split into 3 column
        # groups aligned to the matmul n-slices (PSUM bank boundaries)
        n_slices = [(0, 512), (512, 512), (1024, CK - 1024)]
        cnt_grp = [singles.tile([P, D_CHUNKS, nw], FP8, tag=f"cnt_g{g}",
                                name=f"cnt_g{g}")
                   for g, (n0, nw) in enumerate(n_slices)]

        # ---- phase A: reconstruct + transpose the centers table ----
        if use_cc:
            dram = ctx.enter_context(tc.tile_pool(name="dram", bufs=1, space="DRAM"))
            bounce_in = dram.tile([CK_LOCAL, D], FP8, tag="cc_in")
            bounce_out = dram.tile([CK, D], FP8, tag="cc_out")
            nc.gpsimd.dma_start(out=bounce_in, in_=cnq_dram)
            nc.gpsimd.collective_compute(
                "AllGather",
                mybir.AluOpType.bypass,
                replica_groups=[list(range(N_CORES))],
                ins=[bounce_in.opt()],
                outs=[bounce_out.opt()],
            )
            cn_src = bounce_out
        else:
            cn_src = cnq_dram

        # 12 row-tiles: 11 x 128 rows + 1 x 32 rows, DMAd in 256-row pairs
        groups = [(0, 256), (256, 256), (512, 256), (768, 256),
                  (1024, 256), (1280, 160)]
        for (gr0, grows) in groups:
            nsub = (grows + P - 1) // P
            c_t2 = cpool.tile([P, 2, D], FP8, tag="c_t2")
            if grows % P == 0:
                src = cn_src[gr0:gr0 + grows, :].rearrange(
                    "(two p) d -> p two d", p=P)
                nc.sync.dma_start(out=c_t2[:, :nsub, :], in_=src)
            else:
                nc.sync.dma_start(out=c_t2[:, 0, :],
                                  in_=cn_src[gr0:gr0 + P, :])
                nc.sync.dma_start(out=c_t2[:32, 1, :],
                                  in_=cn_src[gr0 + P:gr0 + grows, :])
            for h in range(nsub):
                r0 = gr0 + h * P
                rn = min(P, CK - r0)
                c_bf = cpool.tile([P, D], BF16, tag="c_bf")
                nc.scalar.activation(out=c_bf[:rn], in_=c_t2[:rn, h, :],
                                     func=mybir.ActivationFunctionType.Copy)
                pt = psum.tile([P, D_CHUNKS * P], BF16, tag="pt")
                for j in range(D_CHUNKS):
                    nc.tensor.transpose(pt[:, j * rn:(j + 1) * rn],
                                        c_bf[:rn, j * P:(j + 1) * P],
                                        ident[:rn, :rn])
                g = (r0 // 512)
                goff = r0 - [0, 512, 1024][g]
                src2 = pt[:, :D_CHUNKS * rn].rearrange("p (j n) -> p j n",
                                                       j=D_CHUNKS)
                nc.vector.tensor_copy(cnt_grp[g][:, :, goff:goff + rn], src2)

        # ---- phase B: per 128-sample tile ----
        for t in range(N_TILES):
            x_t = xpool.tile([P, D], FP8, tag="x_t")
            nc.sync.dma_start(out=x_t, in_=xq_dram[t * P:(t + 1) * P, :])

            # transpose -> xT_sb[p, j*128 + b] = x[b, j*128+p]
            x_bf = xpool.tile([P, D], BF16, tag="x_bf")
            nc.scalar.activation(out=x_bf, in_=x_t,
                                 func=mybir.ActivationFunctionType.Copy)
            pt = psum.tile([P, D_CHUNKS * P], BF16, tag="pt")
            for j in range(D_CHUNKS):
                nc.tensor.transpose(pt[:, j * P:(j + 1) * P],
                                    x_bf[:, j * P:(j + 1) * P], ident)
            xt_sb = xpool.tile([P, D], FP8, tag="xt_sb")
            nc.vector.tensor_copy(xt_sb, pt)

            # S[b, ck] = sum_d x[b,d] cn[ck,d]: DoubleRow, 2 chunks/matmul
            s_ps = psum.tile([P, CK], FP32, tag="s_ps")
            xt_view = xt_sb.rearrange("p (j m) -> p j m", j=D_CHUNKS)
            for g, (n0, nw) in enumerate(n_slices):
                for jp in range(D_CHUNKS // 2):
                    nc.tensor.matmul(s_ps[:, n0:n0 + nw],
                                     xt_view[:, 2 * jp:2 * jp + 2, :],
                                     cnt_grp[g][:, 2 * jp:2 * jp + 2, :],
                                     start=(jp == 0),
                                     stop=(jp == D_CHUNKS // 2 - 1),
                                     perf_mode=mybir.MatmulPerfMode.DoubleRow)

            # one-hot over all 1440 columns: (class_of_col == label)
            ohx = spool.tile([P, CK], BF16, tag="ohx")
            nc.vector.tensor_scalar(out=ohx, in0=colck,
                                    scalar1=labels_sb[:, t:t + 1], scalar2=None,
                                    op0=mybir.AluOpType.is_equal)
            masked = spool.tile([P, CK], BF16, tag="masked")
            nc.vector.tensor_mul(masked, s_ps, ohx)

            # T_raw = rowsum(masked); Q_raw = rowsum(masked^2)
            nc.scalar.activation(out=junk_bf, in_=masked,
                                 func=mybir.ActivationFunctionType.Copy,
                                 accum_out=t_all[:, t:t + 1])
            nc.scalar.activation(out=junk_bf, in_=masked,
                                 func=mybir.ActivationFunctionType.Square,
                                 accum_out=q_all[:, t:t + 1])

        # ---- phase C: tail over [128, 8] ----
        tp = singles
        norm = tp.tile([P, N_TILES], FP32, tag="norm")
        nc.scalar.activation(out=norm, in_=ss_all,
                             func=mybir.ActivationFunctionType.Sqrt,
                             bias=eps_col)
        rinv = tp.tile([P, N_TILES], FP32, tag="rinv")
        nc.vector.reciprocal(out=rinv, in_=norm)
        tn = tp.tile([P, N_TILES], FP32, tag="tn")
        nc.vector.tensor_mul(tn, t_all, rinv)          # T = T_raw / ||x||
        rinv2 = tp.tile([P, N_TILES], FP32, tag="rinv2")
        nc.vector.tensor_mul(rinv2, rinv, rinv)
        qn = tp.tile([P, N_TILES], FP32, tag="qn")
        nc.vector.tensor_mul(qn, q_all, rinv2)         # Q = Q_raw / ||x||^2

        sd = tp.tile([P, N_TILES], FP32, tag="sd")     # sd = 16 - T
        nc.vector.tensor_scalar(out=sd, in0=tn, scalar1=-1.0, scalar2=float(K),
                                op0=mybir.AluOpType.mult, op1=mybir.AluOpType.add)
        ssq = tp.tile([P, N_TILES], FP32, tag="ssq")   # ssq = 16 - 2T + Q
        nc.vector.tensor_scalar(out=ssq, in0=tn, scalar1=-2.0, scalar2=float(K),
                                op0=mybir.AluOpType.mult, op1=mybir.AluOpType.add)
        nc.vector.tensor_add(ssq, ssq, qn)
        rsd = tp.tile([P, N_TILES], FP32, tag="rsd")
        nc.vector.reciprocal(out=rsd, in_=sd)
        ps = tp.tile([P, N_TILES], FP32, tag="ps")     # per_sample = sd - ssq/sd
        nc.vector.tensor_mul(ps, ssq, rsd)
        nc.vector.tensor_sub(ps, sd, ps)

        nc.sync.dma_start(out=out_dram, in_=ps)

    nc.compile()
    return nc


class _Result:
    exec_time_ns = None
    mean_exec_time_ns = None
    max_exec_time_core_id = None

    def __init__(self, results):
        self.results = results


class _Runner:
    def __init__(self, use_cc):
        self.use_cc = use_cc
        self.nc = _build_nc(use_cc)
        install_neuronx_cc_hook()

        in_info = []   # (name, shape, np dtype)
        out_names = []
        out_avals = []
        self.zero_info = []
        for alloc in self.nc.m.functions[0].allocations:
            if not isinstance(alloc, mybir.MemoryLocationSet):
                continue
            name = alloc.memorylocations[0].name
            if alloc.kind == "ExternalInput":
                shape = tuple(alloc.tensor_shape)
                in_info.append((name, shape, mybir.dt.np(alloc.dtype)))
            elif alloc.kind == "ExternalOutput":
                shape = tuple(alloc.tensor_shape)
                npdt = mybir.dt.np(alloc.dtype)
                out_names.append(name)
                out_avals.append(jax.core.ShapedArray(shape, npdt))
                self.zero_info.append((shape, npdt))
        self.in_info = in_info
        self.in_names = [n for (n, _, _) in in_info]
        self.out_names = out_names
        self.out_avals = out_avals

        n_params = len(self.in_names)
        n_outs = len(out_names)
        all_names = tuple(self.in_names + out_names)
        out_avals_t = tuple(out_avals)
        out_names_t = tuple(out_names)
        nc = self.nc

        def _body(*args):
            outs = _bass_exec_p.bind(
                *args,
                out_avals=out_avals_t,
                in_names=all_names,
                out_names=out_names_t,
                lowering_input_output_aliases=(),
                sim_require_finite=True,
                sim_require_nnan=True,
                nc=nc,
            )
            return tuple(outs)

        devices = jax.devices()[:N_CORES]
        assert len(devices) == N_CORES, f"need {N_CORES} devices, got {len(devices)}"
        self.mesh = Mesh(np.asarray(devices), ("core",))
        donate = tuple(range(n_params, n_params + n_outs))
        in_specs = (PSpec("core"),) * (n_params + n_outs)
        out_specs = (PSpec("core"),) * n_outs
        self.sharded = jax.jit(
            shard_map(_body, mesh=self.mesh, in_specs=in_specs,
                      out_specs=out_specs, check_rep=False),
            donate_argnums=donate,
            keep_unused=True,
        )

        # device-resident constants: committed once, zero per-call upload
        sh = NamedSharding(self.mesh, PSpec("core"))
        colck_row = (np.arange(CK, dtype=np.float32) // K).astype(NP_BF16)
        colck_np = np.ascontiguousarray(
            np.broadcast_to(colck_row, (N_CORES * P, CK)))
        ident_np = np.tile(np.eye(P, dtype=NP_BF16), (N_CORES, 1))
        self.const_dev = {
            "colck": jax.device_put(colck_np, sh),
            "ident": jax.device_put(ident_np, sh),
        }

        self.cpu = jax.devices("cpu")[0]

        def _prep(x, labels, centers):
            xq = x.astype(jnp.float8_e4m3)
            ss = jnp.sum(x * x, axis=1)
            ss = ss.reshape(N_CORES, N_TILES, P).transpose(0, 2, 1)
            ss = ss.reshape(N_CORES * P, N_TILES)
            lab = labels.astype(jnp.float32)
            lab = lab.reshape(N_CORES, N_TILES, P).transpose(0, 2, 1)
            lab = lab.reshape(N_CORES * P, N_TILES)
            cn = centers.reshape(CK, D)
            cn = cn * jax.lax.rsqrt(jnp.sum(cn * cn, axis=1, keepdims=True) + EPS)
            cnq = cn.astype(jnp.float8_e4m3)
            if not self.use_cc:
                cnq = jnp.tile(cnq, (N_CORES, 1))
            return xq, lab, ss, cnq

        self._prep = jax.jit(_prep)

        # warm both executables so the first real call is steady-state
        dummy = {
            "x": np.zeros((B, D), np.float32),
            "labels": np.zeros((B,), np.int32),
            "centers": np.ones((C, K, D), np.float32),
        }
        self.execute(**dummy)

    def execute(self, x, labels, centers):
        with jax.default_device(self.cpu):
            xq, lab, ssg, cnq = self._prep(x, labels, centers)
        call_args = {"xq": xq, "labels": lab, "ss": ssg, "cnq": cnq,
                     **self.const_dev}
        args = []
        for (name, shape, npdt) in self.in_info:
            if name in call_args:
                args.append(call_args[name])
            else:
                # internal plumbing tensor (e.g. debug addr): feed zeros
                args.append(np.zeros((N_CORES * shape[0], *shape[1:]), npdt))
        zeros = [np.zeros((N_CORES * s[0], *s[1:]), d)
                 for (s, d) in self.zero_info]
        outs = self.sharded(*args, *zeros)
        out = np.asarray(outs[self.out_names.index("out")], np.float64)
        return np.float32(out.sum() / B)


_RUNNER = None


def _get_runner():
    global _RUNNER
    if _RUNNER is None:
        _RUNNER = _Runner(USE_CC)
    return _RUNNER


def run(x, labels, centers, trace=False, **kw):
    r = _get_runner()
    x = np.ascontiguousarray(np.asarray(x, dtype=np.float32))
    labels = np.asarray(labels).astype(np.int32)
    centers = np.ascontiguousarray(np.asarray(centers, dtype=np.float32))
    loss = r.execute(x, labels, centers)
    return loss, _Result(results=None)


def kernel(x, labels, centers):
    loss, _ = run(x, labels, centers)
    return loss


if not LAZY_INIT:
    try:
        _get_runner()
    except Exception as _e:  # fall back to lazy init on first call
        sys.stderr.write(f"kernel.py: eager init failed ({_e!r}); deferring\n")
        _RUNNER = None


# revision 19
# speedup vs baseline: 1.1364x; 1.1364x over previous
"""Trainium2 Bass kernel for CenterWoParamMultiCosineLoss (l2Norm branch).

Contract: kernel(**inputs) takes FULL inputs (x [8192,1024] f32,
labels [8192] i64/i32, centers [90,16,1024] f32) and returns the FULL
output (scalar f32 loss), running on 8 NeuronCores data-parallel over
the batch.

Math (per sample b, with label c = labels[b], K=16 centers per class):
    xn = x / ||x||;  cn = centers / ||centers||  (rows, +1e-12 under sqrt)
    t_k = xn . cn[c,k]                (16 cosine sims)
    d_k = 1 - t_k
    per_sample = sum_k (1 - d_k/sd) * d_k = sd - ssq/sd
      where sd = sum_k d_k,  ssq = sum_k d_k^2
    loss = mean(per_sample)

End-to-end wall time is dominated by the host->device tunnel, so the
host ships as few bytes as possible:
  - x is cast to fp8e4m3 on host (8 MB total, batch-sharded);
    ||x||^2 is computed on host in exact fp32 and shipped as [128,8]
    per core (tiny).
  - centers are normalized on host, cast to fp8, and sharded 180
    rows/core (1.5 MB total); the device runs an AllGather to
    reconstruct the full 1440-row table on every core.
  - constant tables (colck, ident) are committed to the devices once
    at init and reused every call.
  - the jitted shard_map executable is built once and cached (the
    library path rebuilds it per call).

Device per core (1024 samples):
    - AllGather centers shard -> cn [1440,1024] fp8; PE-transpose into
      the matmul layout cnT [128, 8, 1440] fp8.
    - per 128-sample tile: PE-transpose x tile, 12 DoubleRow fp8
      matmuls S[b, ck] for all 1440 (class,k) columns.
    - masked = S * onehot(label-per-column); T_raw = rowsum(masked),
      Q_raw = rowsum(masked^2) via ACT accum_out.
    - tail: T = T_raw/||x||, Q = Q_raw/||x||^2, per_sample = sd-ssq/sd.
    - host sums the 8x[128,8] per-sample values -> mean.
"""

import os
import sys
from contextlib import ExitStack

import numpy as np

for _p in ("/opt/trn_rl_repo", "/root/.axon_site/_ro/trn_rl_repo"):
    if os.path.isdir(_p) and _p not in sys.path:
        sys.path.insert(0, _p)

import ml_dtypes
import jax
import jax.numpy as jnp
from jax.experimental.shard_map import shard_map
from jax.sharding import Mesh, NamedSharding, PartitionSpec as PSpec

import concourse.bacc as bacc
import concourse.tile as tile
from concourse import mybir
from concourse.bass2jax import (_bass_exec_p, install_neuronx_cc_hook,
                                partition_id_tensor)

N_CORES = 8
B = 8192
B_LOCAL = B // N_CORES  # 1024 samples per core
P = 128                 # partitions
N_TILES = B_LOCAL // P  # 8 sample tiles per core
D = 1024                # feature dim
C = 90                  # classes
K = 16                  # centers per class
CK = C * K              # 1440
CK_LOCAL = CK // N_CORES  # 180 center rows shipped per core
D_CHUNKS = D // P       # 8 contraction chunks
EPS = 1e-12

FP32 = mybir.dt.float32
BF16 = mybir.dt.bfloat16
FP8 = mybir.dt.float8e4
U8 = mybir.dt.uint8
D2 = D // 2

NP_FP8 = ml_dtypes.float8_e4m3
NP_BF16 = ml_dtypes.bfloat16

# AllGather the centers shard on-device (1.5 MB upload) instead of
# replicating the table to all 8 cores from host (12 MB upload).
USE_CC = os.environ.get("BASS_CC", "1") == "1"
LAZY_INIT = os.environ.get("BASS_LAZY", "0") == "1"


def _build_nc(use_cc):
    nc = bacc.Bacc("TRN2", target_bir_lowering=False, debug=False,
                   num_devices=N_CORES)

    # x ships as packed 4-bit: byte b at [row, d] holds q[d] | q[d+512]<<4,
    # q = round(x * 7/max|x_row|) + 8 in [1,15]; the scale is folded into ss
    xq_dram = nc.dram_tensor("xq", [B_LOCAL, D2], U8, kind="ExternalInput").ap()
    labels_dram = nc.dram_tensor("labels", [P, N_TILES], FP32, kind="ExternalInput").ap()
    ss_dram = nc.dram_tensor("ss", [P, N_TILES], FP32, kind="ExternalInput").ap()
    cn_rows = CK_LOCAL if use_cc else CK
    cnq_dram = nc.dram_tensor("cnq", [cn_rows, D], FP8, kind="ExternalInput").ap()
    colck_dram = nc.dram_tensor("colck", [P, CK], BF16, kind="ExternalInput").ap()
    ident_dram = nc.dram_tensor("ident", [P, P], BF16, kind="ExternalInput").ap()
    out_dram = nc.dram_tensor("out", [P, N_TILES], FP32, kind="ExternalOutput").ap()

    with tile.TileContext(nc) as tc, ExitStack() as ctx:
        singles = ctx.enter_context(tc.tile_pool(name="singles", bufs=1))
        cpool = ctx.enter_context(tc.tile_pool(name="cpool", bufs=3))
        xpool = ctx.enter_context(tc.tile_pool(name="xpool", bufs=4))
        spool = ctx.enter_context(tc.tile_pool(name="spool", bufs=3))
        psum = ctx.enter_context(tc.tile_pool(name="psum", bufs=2, space="PSUM"))

        # ---- constants / per-sample stats ----
        ident = singles.tile([P, P], BF16, tag="ident")
        nc.sync.dma_start(out=ident, in_=ident_dram)
        colck = singles.tile([P, CK], BF16, tag="colck")  # class id per S column
        nc.sync.dma_start(out=colck, in_=colck_dram)
        labels_sb = singles.tile([P, N_TILES], FP32, tag="labels_sb")
        nc.sync.dma_start(out=labels_sb, in_=labels_dram)
        ss_all = singles.tile([P, N_TILES], FP32, tag="ss_all")  # sum x^2 (host)
        nc.sync.dma_start(out=ss_all, in_=ss_dram)
        eps_col = singles.tile([P, 1], FP32, tag="eps_col")
        nc.vector.memset(eps_col, EPS)

        t_all = singles.tile([P, N_TILES], FP32, tag="t_all")    # T_raw
        q_all = singles.tile([P, N_TILES], FP32, tag="q_all")    # Q_raw
        junk_bf = singles.tile([P, CK], BF16, tag="junk_bf")

        # persistent transposed-normalized centers, split into 3 column
        # groups aligned to the matmul n-slices (PSUM bank boundaries)
        n_slices = [(0, 512), (512, 512), (1024, CK - 1024)]
        cnt_grp = [singles.tile([P, D_CHUNKS, nw], FP8, tag=f"cnt_g{g}",
                                name=f"cnt_g{g}")
                   for g, (n0, nw) in enumerate(n_slices)]

        # ---- phase A: reconstruct + transpose the centers table ----
        if use_cc:
            dram = ctx.enter_context(tc.tile_pool(name="dram", bufs=1, space="DRAM"))
            bounce_in = dram.tile([CK_LOCAL, D], FP8, tag="cc_in")
            bounce_out = dram.tile([CK, D], FP8, tag="cc_out")
            nc.gpsimd.dma_start(out=bounce_in, in_=cnq_dram)
            nc.gpsimd.collective_compute(
                "AllGather",
                mybir.AluOpType.bypass,
                replica_groups=[list(range(N_CORES))],
                ins=[bounce_in.opt()],
                outs=[bounce_out.opt()],
            )
            cn_src = bounce_out
        else:
            cn_src = cnq_dram

        # 12 row-tiles: 11 x 128 rows + 1 x 32 rows, DMAd in 256-row pairs
        groups = [(0, 256), (256, 256), (512, 256), (768, 256),
                  (1024, 256), (1280, 160)]
        for (gr0, grows) in groups:
            nsub = (grows + P - 1) // P
            c_t2 = cpool.tile([P, 2, D], FP8, tag="c_t2")
            if grows % P == 0:
                src = cn_src[gr0:gr0 + grows, :].rearrange(
                    "(two p) d -> p two d", p=P)
                nc.sync.dma_start(out=c_t2[:, :nsub, :], in_=src)
            else:
                nc.sync.dma_start(out=c_t2[:, 0, :],
                                  in_=cn_src[gr0:gr0 + P, :])
                nc.sync.dma_start(out=c_t2[:32, 1, :],
                                  in_=cn_src[gr0 + P:gr0 + grows, :])
            for h in range(nsub):
                r0 = gr0 + h * P
                rn = min(P, CK - r0)
                c_bf = cpool.tile([P, D], BF16, tag="c_bf")
                nc.scalar.activation(out=c_bf[:rn], in_=c_t2[:rn, h, :],
                                     func=mybir.ActivationFunctionType.Copy)
                pt = psum.tile([P, D_CHUNKS * P], BF16, tag="pt")
                for j in range(D_CHUNKS):
                    nc.tensor.transpose(pt[:, j * rn:(j + 1) * rn],
                                        c_bf[:rn, j * P:(j + 1) * P],
                                        ident[:rn, :rn])
                g = (r0 // 512)
                goff = r0 - [0, 512, 1024][g]
                src2 = pt[:, :D_CHUNKS * rn].rearrange("p (j n) -> p j n",
                                                       j=D_CHUNKS)
                nc.vector.tensor_copy(cnt_grp[g][:, :, goff:goff + rn], src2)

        # ---- phase B: per 128-sample tile ----
        for t in range(N_TILES):
            xp_t = xpool.tile([P, D2], U8, tag="xp_t")
            nc.sync.dma_start(out=xp_t, in_=xq_dram[t * P:(t + 1) * P, :])

            # unpack nibbles -> biased q in bf16 (low -> d<512, high -> rest)
            lo_u8 = xpool.tile([P, D2], U8, tag="lo_u8")
            nc.vector.tensor_scalar(out=lo_u8, in0=xp_t, scalar1=15,
                                    scalar2=None, op0=mybir.AluOpType.bitwise_and)
            hi_u8 = xpool.tile([P, D2], U8, tag="hi_u8")
            nc.vector.tensor_scalar(out=hi_u8, in0=xp_t, scalar1=4,
                                    scalar2=None,
                                    op0=mybir.AluOpType.logical_shift_right)
            x_bf = xpool.tile([P, D], BF16, tag="x_bf")
            nc.vector.tensor_copy(x_bf[:, :D2], lo_u8)
            nc.vector.tensor_copy(x_bf[:, D2:], hi_u8)

            # transpose -> xT_sb[p, j*128 + b] = q[b, j*128+p] - 8
            pt = psum.tile([P, D_CHUNKS * P], BF16, tag="pt")
            for j in range(D_CHUNKS):
                nc.tensor.transpose(pt[:, j * P:(j + 1) * P],
                                    x_bf[:, j * P:(j + 1) * P], ident)
            xt_sb = xpool.tile([P, D], FP8, tag="xt_sb")
            nc.vector.tensor_scalar(out=xt_sb, in0=pt, scalar1=8.0,
                                    scalar2=None, op0=mybir.AluOpType.subtract)

            # S[b, ck] = sum_d x[b,d] cn[ck,d]: DoubleRow, 2 chunks/matmul
            s_ps = psum.tile([P, CK], FP32, tag="s_ps")
            xt_view = xt_sb.rearrange("p (j m) -> p j m", j=D_CHUNKS)
            for g, (n0, nw) in enumerate(n_slices):
                for jp in range(D_CHUNKS // 2):
                    nc.tensor.matmul(s_ps[:, n0:n0 + nw],
                                     xt_view[:, 2 * jp:2 * jp + 2, :],
                                     cnt_grp[g][:, 2 * jp:2 * jp + 2, :],
                                     start=(jp == 0),
                                     stop=(jp == D_CHUNKS // 2 - 1),
                                     perf_mode=mybir.MatmulPerfMode.DoubleRow)

            # one-hot over all 1440 columns: (class_of_col == label)
            ohx = spool.tile([P, CK], BF16, tag="ohx")
            nc.vector.tensor_scalar(out=ohx, in0=colck,
                                    scalar1=labels_sb[:, t:t + 1], scalar2=None,
                                    op0=mybir.AluOpType.is_equal)
            masked = spool.tile([P, CK], BF16, tag="masked")
            nc.vector.tensor_mul(masked, s_ps, ohx)

            # T_raw = rowsum(masked); Q_raw = rowsum(masked^2)
            nc.scalar.activation(out=junk_bf, in_=masked,
                                 func=mybir.ActivationFunctionType.Copy,
                                 accum_out=t_all[:, t:t + 1])
            nc.scalar.activation(out=junk_bf, in_=masked,
                                 func=mybir.ActivationFunctionType.Square,
                                 accum_out=q_all[:, t:t + 1])

        # ---- phase C: tail over [128, 8] ----
        tp = singles
        norm = tp.tile([P, N_TILES], FP32, tag="norm")
        nc.scalar.activation(out=norm, in_=ss_all,
                             func=mybir.ActivationFunctionType.Sqrt,
                             bias=eps_col)
        rinv = tp.tile([P, N_TILES], FP32, tag="rinv")
        nc.vector.reciprocal(out=rinv, in_=norm)
        tn = tp.tile([P, N_TILES], FP32, tag="tn")
        nc.vector.tensor_mul(tn, t_all, rinv)          # T = T_raw / ||x||
        rinv2 = tp.tile([P, N_TILES], FP32, tag="rinv2")
        nc.vector.tensor_mul(rinv2, rinv, rinv)
        qn = tp.tile([P, N_TILES], FP32, tag="qn")
        nc.vector.tensor_mul(qn, q_all, rinv2)         # Q = Q_raw / ||x||^2

        sd = tp.tile([P, N_TILES], FP32, tag="sd")     # sd = 16 - T
        nc.vector.tensor_scalar(out=sd, in0=tn, scalar1=-1.0, scalar2=float(K),
                                op0=mybir.AluOpType.mult, op1=mybir.AluOpType.add)
        ssq = tp.tile([P, N_TILES], FP32, tag="ssq")   # ssq = 16 - 2T + Q
        nc.vector.tensor_scalar(out=ssq, in0=tn, scalar1=-2.0, scalar2=float(K),
                                op0=mybir.AluOpType.mult, op1=mybir.AluOpType.add)
        nc.vector.tensor_add(ssq, ssq, qn)
        rsd = tp.tile([P, N_TILES], FP32, tag="rsd")
        nc.vector.reciprocal(out=rsd, in_=sd)
        ps = tp.tile([P, N_TILES], FP32, tag="ps")     # per_sample = sd - ssq/sd
        nc.vector.tensor_mul(ps, ssq, rsd)
        nc.vector.tensor_sub(ps, sd, ps)

        nc.sync.dma_start(out=out_dram, in_=ps)

    nc.compile()
    return nc


class _Result:
    exec_time_ns = None
    mean_exec_time_ns = None
    max_exec_time_core_id = None

    def __init__(self, results):
        self.results = results


class _Runner:
    def __init__(self, use_cc):
        self.use_cc = use_cc
        self.nc = _build_nc(use_cc)
        install_neuronx_cc_hook()

        partition_name = (self.nc.partition_id_tensor.name
                          if self.nc.partition_id_tensor else None)
        in_info = []   # (name, shape, np dtype)
        out_names = []
        out_avals = []
        self.zero_info = []
        for alloc in self.nc.m.functions[0].allocations:
            if not isinstance(alloc, mybir.MemoryLocationSet):
                continue
            name = alloc.memorylocations[0].name
            if alloc.kind == "ExternalInput":
                if name == partition_name:
                    continue  # supplied in-body via partition_id_tensor()
                shape = tuple(alloc.tensor_shape)
                in_info.append((name, shape, mybir.dt.np(alloc.dtype)))
            elif alloc.kind == "ExternalOutput":
                shape = tuple(alloc.tensor_shape)
                npdt = mybir.dt.np(alloc.dtype)
                out_names.append(name)
                out_avals.append(jax.core.ShapedArray(shape, npdt))
                self.zero_info.append((shape, npdt))
        self.in_info = in_info
        self.in_names = [n for (n, _, _) in in_info]
        self.out_names = out_names
        self.out_avals = out_avals

        n_params = len(self.in_names)
        n_outs = len(out_names)
        all_names = self.in_names + out_names
        if partition_name is not None:
            all_names = all_names + [partition_name]
        all_names = tuple(all_names)
        out_avals_t = tuple(out_avals)
        out_names_t = tuple(out_names)
        nc = self.nc
        has_pid = partition_name is not None

        def _body(*args):
            operands = list(args)
            if has_pid:
                operands.append(partition_id_tensor())
            outs = _bass_exec_p.bind(
                *operands,
                out_avals=out_avals_t,
                in_names=all_names,
                out_names=out_names_t,
                lowering_input_output_aliases=(),
                sim_require_finite=True,
                sim_require_nnan=True,
                nc=nc,
            )
            return tuple(outs)

        devices = jax.devices()[:N_CORES]
        assert len(devices) == N_CORES, f"need {N_CORES} devices, got {len(devices)}"
        self.mesh = Mesh(np.asarray(devices), ("core",))
        donate = tuple(range(n_params, n_params + n_outs))
        in_specs = (PSpec("core"),) * (n_params + n_outs)
        out_specs = (PSpec("core"),) * n_outs
        self.sharded = jax.jit(
            shard_map(_body, mesh=self.mesh, in_specs=in_specs,
                      out_specs=out_specs, check_rep=False),
            donate_argnums=donate,
            keep_unused=True,
        )

        # device-resident constants: committed once, zero per-call upload
        sh = NamedSharding(self.mesh, PSpec("core"))
        colck_row = (np.arange(CK, dtype=np.float32) // K).astype(NP_BF16)
        colck_np = np.ascontiguousarray(
            np.broadcast_to(colck_row, (N_CORES * P, CK)))
        ident_np = np.tile(np.eye(P, dtype=NP_BF16), (N_CORES, 1))
        self.const_dev = {
            "colck": jax.device_put(colck_np, sh),
            "ident": jax.device_put(ident_np, sh),
        }

        self.cpu = jax.devices("cpu")[0]
        self.sh = sh

        def _cast_x(x):
            a = jnp.maximum(jnp.max(jnp.abs(x), axis=1, keepdims=True), 1e-6)
            s = 7.0 / a
            q = jnp.round(x * s).astype(jnp.int32) + 8
            qu = q.astype(jnp.uint8)
            packed = qu[:, :D2] | (qu[:, D2:] << 4)
            # scale-folded ||x||^2 so the device tail needs no extra input:
            # T_raw/sqrt(ss') == (x/s)/||x|| dot products
            ss = jnp.sum(x * x, axis=1, keepdims=True) * (s * s)
            ss = ss.reshape(N_CORES, N_TILES, P).transpose(0, 2, 1)
            ss = ss.reshape(N_CORES * P, N_TILES)
            return packed, ss

        def _prep_rest(labels, centers):
            lab = labels.astype(jnp.float32)
            lab = lab.reshape(N_CORES, N_TILES, P).transpose(0, 2, 1)
            lab = lab.reshape(N_CORES * P, N_TILES)
            cn = centers.reshape(CK, D)
            cn = cn * jax.lax.rsqrt(jnp.sum(cn * cn, axis=1, keepdims=True) + EPS)
            cnq = cn.astype(jnp.float8_e4m3)
            if not self.use_cc:
                cnq = jnp.tile(cnq, (N_CORES, 1))
            return lab, cnq

        self._cast_x = jax.jit(_cast_x)
        self._prep_rest = jax.jit(_prep_rest)

        # warm both executables so the first real call is steady-state
        dummy = {
            "x": np.zeros((B, D), np.float32),
            "labels": np.zeros((B,), np.int32),
            "centers": np.ones((C, K, D), np.float32),
        }
        self.execute(**dummy)

    def execute(self, x, labels, centers):
        # pack x first and launch its (async) upload, then compute the
        # rest of the host prep while the 4 MB stream over the tunnel
        with jax.default_device(self.cpu):
            xq, ssg = self._cast_x(x)
        xq_dev = jax.device_put(xq, self.sh)
        with jax.default_device(self.cpu):
            lab, cnq = self._prep_rest(labels, centers)
        call_args = {"xq": xq_dev, "labels": lab, "ss": ssg, "cnq": cnq,
                     **self.const_dev}
        args = []
        for (name, shape, npdt) in self.in_info:
            if name in call_args:
                args.append(call_args[name])
            else:
                # internal plumbing tensor (e.g. debug addr): feed zeros
                args.append(np.zeros((N_CORES * shape[0], *shape[1:]), npdt))
        zeros = [np.zeros((N_CORES * s[0], *s[1:]), d)
                 for (s, d) in self.zero_info]
        outs = self.sharded(*args, *zeros)
        out = np.asarray(outs[self.out_names.index("out")], np.float64)
        return np.float32(out.sum() / B)


_RUNNER = None


def _get_runner():
    global _RUNNER
    if _RUNNER is None:
        _RUNNER = _Runner(USE_CC)
    return _RUNNER


def run(x, labels, centers, trace=False, **kw):
    r = _get_runner()
    x = np.ascontiguousarray(np.asarray(x, dtype=np.float32))
    labels = np.asarray(labels).astype(np.int32)
    centers = np.ascontiguousarray(np.asarray(centers, dtype=np.float32))
    loss = r.execute(x, labels, centers)
    return loss, _Result(results=None)


def kernel(x, labels, centers):
    loss, _ = run(x, labels, centers)
    return loss


if not LAZY_INIT:
    try:
        _get_runner()
    except Exception as _e:  # fall back to lazy init on first call
        sys.stderr.write(f"kernel.py: eager init failed ({_e!r}); deferring\n")
        _RUNNER = None
